# revision 24
# baseline (speedup 1.0000x reference)
"""Trainium2 Bass kernel for nn_AttentionBlock (Swin-style 7x7 window attention).

Sharding: pure data parallel - batch B=8, one image per NeuronCore; small
weights and the 169x4 relative-bias table replicated (host-folded).

Per-core program (one image, built with Bass/Tile):
- Token order: window-compact, s = 49*w + p; chunk = 128 windows = 6272 tokens.
- Phase A: LN1 (token-major, window-on-partition) -> PE-transpose ->
  feature-major QKV (bf16 matmuls) -> per-window scores with PSUM-accumulated
  relative bias (pad columns = -1e30 so exp()=0) -> ACT exp -> AV matmuls with
  a ones-augmented V (sumexp lands per-token) -> normalize (DVE reciprocal +
  broadcast multiply) -> PE-transpose -> w_out -> residual -> y to DRAM.
- Phase B: LN2 -> FFN (gelu on ACT) -> residual -> scatter back to image order.

Self-contained: shapes/strategy hardcoded; only library imports.
"""
import numpy as np
import ml_dtypes

_CTX = {}
LAST_EXEC_NS = None

B = 8
N = 50176
D = 96
H = 4
DH = 32
HID = 384
EPS = 1e-5
W = 7
SCALE = DH ** -0.5
NEG = -1e30
NCHUNK = 8


def _rel_idx():
    pos = np.arange(W)
    gi, gj = np.meshgrid(pos, pos, indexing="ij")
    grid = np.stack([gi, gj], -1).reshape(-1, 2)
    rel = grid[:, None] - grid[None] + (W - 1)
    return rel[..., 0] * (2 * W - 1) + rel[..., 1]


def _host_consts(w_qkv, w_out, b_out, rel_bias, ln1_g, ln1_b, ln2_g, ln2_b,
                 w1, b1, w2, b2):
    bf = ml_dtypes.bfloat16
    w_qkv = np.asarray(w_qkv, np.float32)
    wq, wk, wv = w_qkv[0:128], w_qkv[128:256], w_qkv[256:384]
    g1 = np.asarray(ln1_g, np.float32)
    b1v = np.asarray(ln1_b, np.float32)

    def aug(wmat, gamma, beta, extra_scale=1.0):
        out = np.zeros((97, wmat.shape[0]), np.float32)
        out[0:96] = (wmat * gamma[None, :] * extra_scale).T
        out[96] = (wmat * extra_scale) @ beta
        return out

    wqT = aug(wq, g1, b1v, SCALE)
    wkT = aug(wk, g1, b1v)
    wvT_c = aug(wv, g1, b1v)
    wv_augT = np.zeros((97, 132), np.float32)
    for h in range(H):
        wv_augT[:, 33 * h:33 * h + 32] = wvT_c[:, 32 * h:32 * h + 32]
        wv_augT[96, 33 * h + 32] = 1.0
    rb = np.asarray(rel_bias, np.float32)
    bias_h = rb[_rel_idx()].transpose(2, 0, 1) * SCALE
    bias_tbl = np.full((49, H, 64), NEG, np.float32)
    for h in range(H):
        bias_tbl[:, h, 0:49] = bias_h[h]
    g2 = np.asarray(ln2_g, np.float32)
    b2v = np.asarray(ln2_b, np.float32)
    w1m = np.asarray(w1, np.float32)
    w1_augT = np.zeros((97, HID), np.float32)
    w1_augT[0:96] = (w1m * g2[None, :]).T
    w1_augT[96] = w1m @ b2v + np.asarray(b1, np.float32)
    w2T = np.asarray(w2, np.float32).T.reshape(3, 128, 96).transpose(1, 0, 2)
    c = {
        "wqT": wqT, "wkT": wkT, "wv_augT": wv_augT,
        "bias_tbl": bias_tbl, "eye49": np.eye(49, dtype=np.float32),
        "w_outT": np.asarray(w_out, np.float32).T,
        "b_out": np.asarray(b_out, np.float32).reshape(96, 1),
        "w1_augT": w1_augT, "w2T": w2T,
        "b2": np.asarray(b2, np.float32).reshape(96, 1),
        "ones_row": np.ones((1, 8192), np.float32),
    }
    return {k: (v.astype(np.float32) if k in ("b_out", "b2") else v.astype(bf))
            for k, v in c.items()}


def _split_multiwaits(nc, max_waits=1):
    """Walrus here allows 1 sync-wait per instruction; Tile emits multi-wait
    instructions. Split extras onto same-engine nops inserted just before."""
    import bass_rust

    def make_nop(eng):
        if hasattr(eng, "nop"):
            try:
                bi = eng.nop()
                return bi.ins if hasattr(bi, "ins") else bi
            except (AttributeError, TypeError):
                pass
        bi = eng.engine_nop()
        return bi.ins if hasattr(bi, "ins") else bi

    engines = {}
    for name in ("tensor", "vector", "scalar", "gpsimd", "sync"):
        eng = getattr(nc, name)
        engines[eng.engine] = eng
    n_split = 0
    for bbname, bbw in list(nc.bb_map.items()):
        lst = bbw.bb.instructions
        k = 0
        while k < len(lst):
            inst = lst[k]
            si = inst.sync_info
            if si is None:
                k += 1
                continue
            waits = list(si.on_wait or [])
            if len(waits) > max_waits:
                eng = engines.get(inst.engine)
                if eng is None:
                    k += 1
                    continue
                extra, keep = waits[:-max_waits], waits[-max_waits:]
                nops = []
                for w in extra:
                    nop_inst = make_nop(eng)
                    for obbw in nc.bb_map.values():
                        ol = obbw.bb.instructions
                        removed = False
                        for j in range(len(ol) - 1, -1, -1):
                            if ol[j] is nop_inst:
                                ol.pop(j)
                                removed = True
                                break
                        if removed:
                            break
                    nop_inst.sync_info = bass_rust.SyncInfo(on_wait=[w],
                                                            on_update=[])
                    nops.append(nop_inst)
                si.on_wait = keep
                inst.sync_info = si
                for j, nop_inst in enumerate(nops):
                    lst.insert(k + j, nop_inst)
                k += len(nops)
                n_split += 1
            k += 1
    return n_split


def _build(weights, n_chunks=NCHUNK):
    import concourse.bass as bass
    import concourse.tile as tile
    from concourse import mybir

    F32 = mybir.dt.float32
    BF16 = mybir.dt.bfloat16
    AF = mybir.ActivationFunctionType
    OP = mybir.AluOpType
    AX = mybir.AxisListType

    consts = _host_consts(**weights)
    nc = bass.Bass()
    xin = nc.declare_dram_parameter("x", [N, D], F32, isOutput=False)
    out_d = nc.declare_dram_parameter("out", [N, D], F32, isOutput=True)
    y_d = nc.dram_tensor("y_buf", [1024, 49, D], F32)
    cd = {}
    for k, v in consts.items():
        dt = F32 if v.dtype == np.float32 else BF16
        cd[k] = nc.declare_dram_parameter(k, list(v.shape), dt, isOutput=False)

    def dram_win_ap(t, chunk, wr_l):
        wr = 4 * chunk + wr_l
        base = (224 * 7 * wr) * 96
        return bass.AP(tensor=t, offset=base,
                       ap=[[7 * 96, 32], [224 * 96, 7], [1, 7 * 96]])

    with tile.TileContext(nc) as tc:
        pools = []

        def pool(name, bufs, space="SBUF"):
            p = tc.alloc_tile_pool(name=name, bufs=bufs, space=space)
            pools.append(p)
            return p

        kp = pool("konst", 1)
        sb_c = {}
        for k, darr in cd.items():
            if k == "ones_row":
                continue
            t = kp.tile(list(darr.shape), darr.dtype, name=f"c_{k}",
                        tag=f"c_{k}")
            nc.sync.dma_start(out=t,
                              in_=darr[(slice(None),) * len(darr.shape)])
            sb_c[k] = t

        xp = pool("xp", 2)
        statp = pool("statp", 1)
        lnp = pool("lnp", 1)
        ltp = pool("ltp", 1)
        qkp = pool("qkp", 1)
        vap = pool("vap", 32)
        exp_p = pool("exp_p", 2)
        onp = pool("onp", 1)
        otp = pool("otp", 1)
        aop = pool("aop", 1)
        yp = pool("yp", 1)
        tokp = pool("tokp", 1)
        ln2p = pool("ln2p", 2)
        h1p = pool("h1p", 1)
        outp = pool("outp", 1)

        ps_s = pool("ps_s", 2, space="PSUM")
        ps_a = pool("ps_a", 2, space="PSUM")
        ps_m = pool("ps_m", 2, space="PSUM")

        QBOUNDS = ((0, 25), (25, 49))

        def ln_layer(src, name):
            """src [128, 49, 96] f32 -> [128, 49, 128] bf16, streamed in
            position-quarters so consumers can start early."""
            lnt = lnp.tile([128, 49, 128], BF16, name=f"ln_{name}", tag="lnt2")
            for qi, (p0, p1) in enumerate(QBOUNDS):
                np_ = p1 - p0
                sums = statp.tile([128, 25], F32, name=f"sm_{name}{qi}",
                                  tag="sums")
                nc.vector.tensor_reduce(out=sums[:, 0:np_],
                                        in_=src[:, p0:p1, :],
                                        axis=AX.X, op=OP.add)
                sq = lnp.tile([128, 25, 96], F32, name=f"sq_{name}{qi}",
                              tag="sq")
                nc.vector.tensor_mul(sq[:, 0:np_, :], src[:, p0:p1, :],
                                     src[:, p0:p1, :])
                sumsq = statp.tile([128, 25], F32, name=f"s2_{name}{qi}",
                                   tag="sumsq")
                nc.vector.tensor_reduce(out=sumsq[:, 0:np_],
                                        in_=sq[:, 0:np_, :],
                                        axis=AX.X, op=OP.add)
                m = statp.tile([128, 25], F32, name=f"m_{name}{qi}", tag="m")
                nc.vector.tensor_scalar_mul(m[:, 0:np_], sums[:, 0:np_],
                                            1.0 / 96)
                m2 = statp.tile([128, 25], F32, name=f"m2_{name}{qi}",
                                tag="m2")
                nc.vector.tensor_mul(m2[:, 0:np_], m[:, 0:np_], m[:, 0:np_])
                veps = statp.tile([128, 25], F32, name=f"ve_{name}{qi}",
                                  tag="veps")
                nc.vector.scalar_tensor_tensor(
                    out=veps[:, 0:np_], in0=sumsq[:, 0:np_],
                    scalar=1.0 / 96, op0=OP.mult,
                    in1=m2[:, 0:np_], op1=OP.subtract)
                nc.vector.tensor_scalar_add(veps[:, 0:np_], veps[:, 0:np_],
                                            EPS)
                lnv = statp.tile([128, 25], F32, name=f"lv_{name}{qi}",
                                 tag="lnv")
                nc.scalar.activation(out=lnv[:, 0:np_], in_=veps[:, 0:np_],
                                     func=AF.Ln)
                rstd = statp.tile([128, 25], F32, name=f"rs_{name}{qi}",
                                  tag="rstd")
                nc.scalar.activation(out=rstd[:, 0:np_], in_=lnv[:, 0:np_],
                                     func=AF.Exp, scale=-0.5)
                nc.vector.scalar_tensor_tensor(
                    out=lnt[:, p0:p1, 0:96], in0=src[:, p0:p1, :],
                    scalar=1.0, op0=OP.mult,
                    in1=m[:, 0:np_, None].to_broadcast([128, np_, 96]),
                    op1=OP.subtract)
                nc.vector.scalar_tensor_tensor(
                    out=lnt[:, p0:p1, 0:96], in0=lnt[:, p0:p1, 0:96],
                    scalar=1.0, op0=OP.mult,
                    in1=rstd[:, 0:np_, None].to_broadcast([128, np_, 96]),
                    op1=OP.mult)
            return lnt

        def transpose_to_T(lnt, dstT):
            """Quartered DMA block-transposes: lnt [128, 49, 128] ->
            dstT[:, 0:6272] position-major."""
            for (p0, p1) in QBOUNDS:
                nc.sync.dma_start_transpose(
                    dstT[:, 128 * p0:128 * p1].rearrange(
                        "a (p w) -> a p w", w=128),
                    lnt[:, p0:p1, :].rearrange("a b c -> a (b c)"))

        def transpose_to_tok(srcT, dst_tok):
            """DMA block-transpose: srcT [96, 6272] -> dst_tok [128, 49, 96]."""
            nc.sync.dma_start_transpose(dst_tok, srcT[0:96, 0:6272])

        eye_b = sb_c["eye49"][:, None, :].to_broadcast([49, 8, 49])

        # ---------------- PHASE A ----------------
        st_a = {}

        def prep_a(c):
            x_tok = xp.tile([128, 49, 96], F32, name="x_tok", tag="x_tok")
            for wr_l in range(4):
                nc.sync.dma_start(out=x_tok[32 * wr_l:32 * wr_l + 32, :, :],
                                  in_=dram_win_ap(xin, c, wr_l))
            ln_x = ln_layer(x_tok, f"a{c}")
            ln_xT = ltp.tile([128, 8192], BF16, name="ln_xT", tag="ln_xT")
            nc.gpsimd.memset(ln_xT[0:97, 6272:8192], 0.0)
            transpose_to_T(ln_x, ln_xT)
            nc.sync.dma_start(out=ln_xT[96:97, 0:6272],
                              in_=cd["ones_row"][0:1, 0:6272])
            st_a[c] = (x_tok, ln_xT)

        def phase_a(c):
            x_tok, ln_xT = st_a.pop(c)
            ln_xT_w = ln_xT.rearrange("k (p w) -> k w p", w=128)

            qT = qkp.tile([128, 6272], BF16, name="qT", tag="qT")
            kT = qkp.tile([128, 6272], BF16, name="kT", tag="kT")
            for n0 in range(0, 6272, 448):
                qps = ps_m.tile([128, 448], F32, name="qps", tag="mm")
                nc.tensor.matmul(qps, sb_c["wqT"][0:97, :],
                                 ln_xT[0:97, n0:n0 + 448],
                                 start=True, stop=True)
                nc.vector.tensor_copy(out=qT[:, n0:n0 + 448], in_=qps)
                kps = ps_m.tile([128, 448], F32, name="kps", tag="mm")
                nc.tensor.matmul(kps, sb_c["wkT"][0:97, :],
                                 ln_xT[0:97, n0:n0 + 448],
                                 start=True, stop=True)
                nc.scalar.copy(out=kT[:, n0:n0 + 448], in_=kps)

            qT_w = qT.rearrange("k (p w) -> k w p", w=128)
            kT_w = kT.rearrange("k (p w) -> k w p", w=128)

            attn_T = aop.tile([96, 6272], BF16, name="attn_T", tag="attn_T")
            attn_T_w = attn_T.rearrange("a (p w) -> a p w", w=128)

            v_tiles = []
            for vt in range(32):
                vps = ps_a.tile([128, 264], F32, name="vps", tag="av")
                for dpq in range(2):
                    for par in range(2):
                        wloc = 2 * (2 * vt + dpq) + par
                        nc.tensor.matmul(
                            vps[64 * par:64 * par + 64,
                                132 * dpq:132 * dpq + 132],
                            ln_xT_w[0:97, wloc, :],
                            sb_c["wv_augT"][0:97, :],
                            start=True, stop=True, skip_group_check=True)
                v_sb = vap.tile([128, 264], BF16, name="v_sb", tag="v_sb")
                nc.scalar.copy(out=v_sb, in_=vps)
                v_tiles.append(v_sb)

            for g in range(8):
                w0 = 16 * g
                exp_ts = []
                for hh in (0, 2):
                    sps = ps_s.tile([128, 1024], F32, name="sps", tag="sps")
                    sps_r = sps.rearrange("a (r x) -> a r x", x=512)
                    for r in range(2):
                        h = hh + r
                        for par in range(2):
                            nc.tensor.matmul(
                                sps_r[64 * par:64 * par + 64, r, 0:392],
                                sb_c["bias_tbl"][:, h, :], eye_b,
                                start=True, stop=False, skip_group_check=True)
                        for j in range(8):
                            for par in range(2):
                                wloc = w0 + 2 * j + par
                                nc.tensor.matmul(
                                    sps_r[64 * par:64 * par + 49, r,
                                          49 * j:49 * j + 49],
                                    kT_w[32 * h:32 * h + 32, wloc, 0:49],
                                    qT_w[32 * h:32 * h + 32, wloc, 0:49],
                                    start=False, stop=True,
                                    tile_position=(32 * h, 64 * par),
                                    skip_group_check=True)
                    exp_sb = exp_p.tile([128, 2, 392], BF16, name="exp_sb",
                                        tag="exp_sb")
                    nc.scalar.activation(out=exp_sb, in_=sps_r[:, :, 0:392],
                                         func=AF.Exp)
                    exp_ts.append(exp_sb)

                o_norm_g = onp.tile([128, 1024], BF16, name="o_norm_g",
                                    tag="o_norm")
                for q2 in range(4):
                    avp = ps_a.tile([128, 264], F32, name="avp", tag="av")
                    avp_b = avp.rearrange("a (b c) -> a b c", c=33)
                    for dpq in range(2):
                        j = 2 * q2 + dpq
                        for h in range(4):
                            e_sb = exp_ts[h // 2]
                            r = h % 2
                            for par in range(2):
                                nc.tensor.matmul(
                                    avp[64 * par:64 * par + 49,
                                        132 * dpq + 33 * h:
                                        132 * dpq + 33 * h + 33],
                                    e_sb[64 * par:64 * par + 64, r,
                                         49 * j:49 * j + 49],
                                    v_tiles[4 * g + q2][64 * par:64 * par + 64,
                                                132 * dpq + 33 * h:
                                                132 * dpq + 33 * h + 33],
                                    start=True, stop=True,
                                    skip_group_check=True)
                    rc = statp.tile([128, 8], F32, name="rc", tag="rc")
                    nc.vector.reciprocal(
                        rc.rearrange("a (b c) -> a b c", c=1),
                        avp_b[:, :, 32:33])
                    nc.vector.scalar_tensor_tensor(
                        out=o_norm_g[:, 256 * q2:256 * q2 + 256].rearrange(
                            "a (b c) -> a b c", c=32),
                        in0=avp_b[:, :, 0:32], scalar=1.0, op0=OP.mult,
                        in1=rc[:, :, None].to_broadcast([128, 8, 32]),
                        op1=OP.mult)

                # one block-transpose + two 4-pair w_out matmuls per group
                oT_sb = otp.tile([128, 1024], BF16, name="oT_sb", tag="oT_sb")
                nc.sync.dma_start_transpose(
                    oT_sb.rearrange("a (r c) -> a r c", c=128),
                    o_norm_g[:, :])
                for half in range(2):
                    wop = ps_m.tile([96, 392], F32, name="wop", tag="mm")
                    nc.tensor.matmul(
                        wop, sb_c["w_outT"],
                        oT_sb.rearrange("a (r m q) -> a r m q",
                                        r=8, q=64)[:, 4 * half:4 * half + 4,
                                                   :, 0:49],
                        start=True, stop=True)
                    wbase = 2 * (8 * g + 4 * half)
                    nc.vector.tensor_scalar_add(
                        attn_T_w[:, :, wbase:wbase + 8].transpose([0, 2, 1]),
                        wop.rearrange("a (r m q) -> a (r m) q", r=4, m=2),
                        sb_c["b_out"][0:96, 0:1])

            for (p0, p1) in ((0, 28), (28, 49)):
                attn_tok = tokp.tile([128, 28, 96], BF16, name="attn_tok",
                                     tag="tok_b")
                nc.sync.dma_start_transpose(
                    attn_tok[:, 0:p1 - p0, :],
                    attn_T[0:96, 128 * p0:128 * p1])
                y_tok = yp.tile([128, 28, 96], F32, name="y_tok", tag="y_tok")
                nc.vector.scalar_tensor_tensor(
                    out=y_tok[:, 0:p1 - p0, :], in0=attn_tok[:, 0:p1 - p0, :],
                    scalar=1.0, op0=OP.mult,
                    in1=x_tok[:, p0:p1, :], op1=OP.add)
                nc.sync.dma_start(out=y_d[128 * c:128 * c + 128, p0:p1, :],
                                  in_=y_tok[:, 0:p1 - p0, :])

        # ---------------- PHASE B ----------------
        st_b = {}

        def prep_b(c):
            y_in = xp.tile([128, 49, 96], F32, name="y_in", tag="x_tok")
            nc.sync.dma_start(out=y_in, in_=y_d[128 * c:128 * c + 128, :, :])
            ln2 = ln_layer(y_in, f"b{c}")
            ln2T = ln2p.tile([128, 6272], BF16, name="ln2T", tag="ln2T")
            transpose_to_T(ln2, ln2T)
            nc.sync.dma_start(out=ln2T[96:97, 0:6272],
                              in_=cd["ones_row"][0:1, 0:6272])
            st_b[c] = (y_in, ln2T)

        def phase_b(c):
            y_in, ln2T = st_b.pop(c)
            ffn_T = aop.tile([96, 6272], BF16, name="ffn_T", tag="ffn_T")
            for n0 in range(0, 6272, 448):
                h1 = h1p.tile([128, 3, 448], BF16, name="h1", tag="h1")
                for m in range(3):
                    fps = ps_m.tile([128, 448], F32, name="fps", tag="mm")
                    nc.tensor.matmul(
                        fps, sb_c["w1_augT"][0:97, 128 * m:128 * m + 128],
                        ln2T[0:97, n0:n0 + 448], start=True, stop=True)
                    nc.scalar.activation(out=h1[:, m, :], in_=fps,
                                         func=AF.Gelu)
                f2 = ps_m.tile([96, 448], F32, name="f2", tag="mm")
                for m in range(3):
                    nc.tensor.matmul(f2, sb_c["w2T"][:, m, :], h1[:, m, :],
                                     start=(m == 0), stop=(m == 2),
                                     skip_group_check=True)
                if (n0 // 448) % 2 == 0:
                    nc.vector.tensor_scalar_add(ffn_T[0:96, n0:n0 + 448], f2,
                                                sb_c["b2"][0:96, 0:1])
                else:
                    nc.scalar.activation(out=ffn_T[0:96, n0:n0 + 448], in_=f2,
                                         func=AF.Identity,
                                         bias=sb_c["b2"][0:96, 0:1], scale=1.0)

            for (p0, p1) in ((0, 28), (28, 49)):
                ffn_tok = tokp.tile([128, 28, 96], BF16, name="ffn_tok",
                                    tag="tok_b")
                nc.sync.dma_start_transpose(
                    ffn_tok[:, 0:p1 - p0, :],
                    ffn_T[0:96, 128 * p0:128 * p1])
                out_tok = outp.tile([128, 28, 96], F32, name="out_tok",
                                    tag="out_tok")
                nc.vector.scalar_tensor_tensor(
                    out=out_tok[:, 0:p1 - p0, :],
                    in0=ffn_tok[:, 0:p1 - p0, :], scalar=1.0, op0=OP.mult,
                    in1=y_in[:, p0:p1, :], op1=OP.add)
                for wr_l in range(4):
                    wr = 4 * c + wr_l
                    base = (224 * 7 * wr + 224 * (p0 // 7)) * 96
                    dst = bass.AP(tensor=out_d, offset=base,
                                  ap=[[7 * 96, 32],
                                      [224 * 96, (p1 - p0) // 7],
                                      [1, 7 * 96]])
                    nc.sync.dma_start(
                        out=dst,
                        in_=out_tok[32 * wr_l:32 * wr_l + 32, 0:p1 - p0, :])

        prep_a(0)
        for c in range(n_chunks):
            if c + 1 < n_chunks:
                prep_a(c + 1)
            phase_a(c)
        prep_b(0)
        for c in range(n_chunks):
            if c + 1 < n_chunks:
                prep_b(c + 1)
            phase_b(c)

        for p in reversed(pools):
            p.release()

    _split_multiwaits(nc)
    return nc, consts


def kernel(x, w_qkv, w_out, b_out, rel_bias, ln1_g, ln1_b, ln2_g, ln2_b,
           w1, b1, w2, b2):
    from concourse.bass_utils import run_bass_kernel_spmd

    if "nc" not in _CTX:
        weights = dict(w_qkv=w_qkv, w_out=w_out, b_out=b_out,
                       rel_bias=rel_bias, ln1_g=ln1_g, ln1_b=ln1_b,
                       ln2_g=ln2_g, ln2_b=ln2_b, w1=w1, b1=b1, w2=w2, b2=b2)
        nc, consts = _build(weights)
        _CTX["nc"] = nc
        _CTX["consts"] = consts

    x = np.asarray(x, np.float32)
    in_maps = []
    for i in range(B):
        m = {"x": np.ascontiguousarray(x[i])}
        m.update(_CTX["consts"])
        in_maps.append(m)
    _CTX["in_maps"] = in_maps

    res = run_bass_kernel_spmd(_CTX["nc"], in_maps, core_ids=list(range(B)))
    global LAST_EXEC_NS
    if res.exec_time_ns:
        LAST_EXEC_NS = res.exec_time_ns
    out = np.stack([np.asarray(res.results[i]["out"]) for i in range(B)])
    return out.astype(np.float32)


# revision 25
# speedup vs baseline: 1.1911x; 1.1911x over previous
"""Trainium2 Bass kernel for nn_AttentionBlock (Swin-style 7x7 window attention).

Sharding: pure data parallel - batch B=8, one image per NeuronCore; small
weights and the 169x4 relative-bias table replicated (host-folded).

Per-core program (one image, built with Bass/Tile):
- Token order: window-compact, s = 49*w + p; chunk = 128 windows = 6272 tokens.
- Phase A: LN1 (token-major, window-on-partition) -> PE-transpose ->
  feature-major QKV (bf16 matmuls) -> per-window scores with PSUM-accumulated
  relative bias (pad columns = -1e30 so exp()=0) -> ACT exp -> AV matmuls with
  a ones-augmented V (sumexp lands per-token) -> normalize (DVE reciprocal +
  broadcast multiply) -> PE-transpose -> w_out -> residual -> y to DRAM.
- Phase B: LN2 -> FFN (gelu on ACT) -> residual -> scatter back to image order.

Self-contained: shapes/strategy hardcoded; only library imports.
"""
import numpy as np
import ml_dtypes

_CTX = {}
LAST_EXEC_NS = None

B = 8
N = 50176
D = 96
H = 4
DH = 32
HID = 384
EPS = 1e-5
W = 7
SCALE = DH ** -0.5
NEG = -1e30
NCHUNK = 8


def _rel_idx():
    pos = np.arange(W)
    gi, gj = np.meshgrid(pos, pos, indexing="ij")
    grid = np.stack([gi, gj], -1).reshape(-1, 2)
    rel = grid[:, None] - grid[None] + (W - 1)
    return rel[..., 0] * (2 * W - 1) + rel[..., 1]


def _host_consts(w_qkv, w_out, b_out, rel_bias, ln1_g, ln1_b, ln2_g, ln2_b,
                 w1, b1, w2, b2):
    bf = ml_dtypes.bfloat16
    w_qkv = np.asarray(w_qkv, np.float32)
    wq, wk, wv = w_qkv[0:128], w_qkv[128:256], w_qkv[256:384]
    g1 = np.asarray(ln1_g, np.float32)
    b1v = np.asarray(ln1_b, np.float32)

    def aug(wmat, gamma, beta, extra_scale=1.0):
        out = np.zeros((97, wmat.shape[0]), np.float32)
        out[0:96] = (wmat * gamma[None, :] * extra_scale).T
        out[96] = (wmat * extra_scale) @ beta
        return out

    wqT = aug(wq, g1, b1v, SCALE)
    wkT = aug(wk, g1, b1v)
    wvT_c = aug(wv, g1, b1v)
    wv_augT = np.zeros((97, 132), np.float32)
    for h in range(H):
        wv_augT[:, 33 * h:33 * h + 32] = wvT_c[:, 32 * h:32 * h + 32]
        wv_augT[96, 33 * h + 32] = 1.0
    rb = np.asarray(rel_bias, np.float32)
    bias_h = rb[_rel_idx()].transpose(2, 0, 1) * SCALE
    bias_tbl = np.full((49, H, 64), NEG, np.float32)
    for h in range(H):
        bias_tbl[:, h, 0:49] = bias_h[h]
    g2 = np.asarray(ln2_g, np.float32)
    b2v = np.asarray(ln2_b, np.float32)
    w1m = np.asarray(w1, np.float32)
    w1_augT = np.zeros((97, HID), np.float32)
    w1_augT[0:96] = (w1m * g2[None, :]).T
    w1_augT[96] = w1m @ b2v + np.asarray(b1, np.float32)
    w2T = np.asarray(w2, np.float32).T.reshape(3, 128, 96).transpose(1, 0, 2)
    c = {
        "wqT": wqT, "wkT": wkT, "wv_augT": wv_augT,
        "bias_tbl": bias_tbl, "eye49": np.eye(49, dtype=np.float32),
        "w_outT": np.asarray(w_out, np.float32).T,
        "b_out": np.asarray(b_out, np.float32).reshape(96, 1),
        "w1_augT": w1_augT, "w2T": w2T,
        "b2": np.asarray(b2, np.float32).reshape(96, 1),
        "ones_row": np.ones((1, 8192), np.float32),
    }
    return {k: (v.astype(np.float32) if k in ("b_out", "b2") else v.astype(bf))
            for k, v in c.items()}


def _split_multiwaits(nc, max_waits=1):
    """Walrus here allows 1 sync-wait per instruction; Tile emits multi-wait
    instructions. Split extras onto same-engine nops inserted just before."""
    import bass_rust

    def make_nop(eng):
        if hasattr(eng, "nop"):
            try:
                bi = eng.nop()
                return bi.ins if hasattr(bi, "ins") else bi
            except (AttributeError, TypeError):
                pass
        bi = eng.engine_nop()
        return bi.ins if hasattr(bi, "ins") else bi

    engines = {}
    for name in ("tensor", "vector", "scalar", "gpsimd", "sync"):
        eng = getattr(nc, name)
        engines[eng.engine] = eng
    n_split = 0
    for bbname, bbw in list(nc.bb_map.items()):
        lst = bbw.bb.instructions
        k = 0
        while k < len(lst):
            inst = lst[k]
            si = inst.sync_info
            if si is None:
                k += 1
                continue
            waits = list(si.on_wait or [])
            if len(waits) > max_waits:
                eng = engines.get(inst.engine)
                if eng is None:
                    k += 1
                    continue
                extra, keep = waits[:-max_waits], waits[-max_waits:]
                nops = []
                for w in extra:
                    nop_inst = make_nop(eng)
                    for obbw in nc.bb_map.values():
                        ol = obbw.bb.instructions
                        removed = False
                        for j in range(len(ol) - 1, -1, -1):
                            if ol[j] is nop_inst:
                                ol.pop(j)
                                removed = True
                                break
                        if removed:
                            break
                    nop_inst.sync_info = bass_rust.SyncInfo(on_wait=[w],
                                                            on_update=[])
                    nops.append(nop_inst)
                si.on_wait = keep
                inst.sync_info = si
                for j, nop_inst in enumerate(nops):
                    lst.insert(k + j, nop_inst)
                k += len(nops)
                n_split += 1
            k += 1
    return n_split


def _build(weights, n_chunks=NCHUNK):
    import concourse.bass as bass
    import concourse.tile as tile
    from concourse import mybir

    F32 = mybir.dt.float32
    BF16 = mybir.dt.bfloat16
    AF = mybir.ActivationFunctionType
    OP = mybir.AluOpType
    AX = mybir.AxisListType

    consts = _host_consts(**weights)
    nc = bass.Bass()
    xin = nc.declare_dram_parameter("x", [N, D], F32, isOutput=False)
    out_d = nc.declare_dram_parameter("out", [N, D], F32, isOutput=True)
    y_d = nc.dram_tensor("y_buf", [1024, 49, D], F32)
    cd = {}
    for k, v in consts.items():
        dt = F32 if v.dtype == np.float32 else BF16
        cd[k] = nc.declare_dram_parameter(k, list(v.shape), dt, isOutput=False)

    def dram_win_ap(t, chunk, wr_l):
        wr = 4 * chunk + wr_l
        base = (224 * 7 * wr) * 96
        return bass.AP(tensor=t, offset=base,
                       ap=[[7 * 96, 32], [224 * 96, 7], [1, 7 * 96]])

    with tile.TileContext(nc) as tc:
        pools = []

        def pool(name, bufs, space="SBUF"):
            p = tc.alloc_tile_pool(name=name, bufs=bufs, space=space)
            pools.append(p)
            return p

        kp = pool("konst", 1)
        sb_c = {}
        for k, darr in cd.items():
            if k == "ones_row":
                continue
            t = kp.tile(list(darr.shape), darr.dtype, name=f"c_{k}",
                        tag=f"c_{k}")
            nc.sync.dma_start(out=t,
                              in_=darr[(slice(None),) * len(darr.shape)])
            sb_c[k] = t

        xp = pool("xp", 2)
        statp = pool("statp", 2)
        lnp = pool("lnp", 1)
        ltp = pool("ltp", 1)
        qkp = pool("qkp", 1)
        vap = pool("vap", 32)
        exp_p = pool("exp_p", 2)
        onp = pool("onp", 1)
        otp = pool("otp", 2)
        aop = pool("aop", 1)
        yp = pool("yp", 1)
        tokp = pool("tokp", 1)
        h1p = pool("h1p", 2)
        outp = pool("outp", 1)

        ps_s = pool("ps_s", 2, space="PSUM")
        ps_a = pool("ps_a", 2, space="PSUM")
        ps_m = pool("ps_m", 2, space="PSUM")

        QBOUNDS = ((0, 49),)

        def ln_layer(src, name):
            """src [128, 49, 96] f32 -> [128, 49, 128] bf16, streamed in
            position-quarters so consumers can start early."""
            lnt = lnp.tile([128, 49, 128], BF16, name=f"ln_{name}", tag="lnt2")
            for qi, (p0, p1) in enumerate(QBOUNDS):
                np_ = p1 - p0
                sums = statp.tile([128, 49], F32, name=f"sm_{name}{qi}",
                                  tag="sums")
                nc.vector.tensor_reduce(out=sums[:, 0:np_],
                                        in_=src[:, p0:p1, :],
                                        axis=AX.X, op=OP.add)
                sq = lnp.tile([128, 49, 96], F32, name=f"sq_{name}{qi}",
                              tag="sq")
                nc.vector.tensor_mul(sq[:, 0:np_, :], src[:, p0:p1, :],
                                     src[:, p0:p1, :])
                sumsq = statp.tile([128, 49], F32, name=f"s2_{name}{qi}",
                                   tag="sumsq")
                nc.vector.tensor_reduce(out=sumsq[:, 0:np_],
                                        in_=sq[:, 0:np_, :],
                                        axis=AX.X, op=OP.add)
                m = statp.tile([128, 49], F32, name=f"m_{name}{qi}", tag="m")
                nc.vector.tensor_scalar_mul(m[:, 0:np_], sums[:, 0:np_],
                                            1.0 / 96)
                m2 = statp.tile([128, 49], F32, name=f"m2_{name}{qi}",
                                tag="m2")
                nc.vector.tensor_mul(m2[:, 0:np_], m[:, 0:np_], m[:, 0:np_])
                veps = statp.tile([128, 49], F32, name=f"ve_{name}{qi}",
                                  tag="veps")
                nc.vector.scalar_tensor_tensor(
                    out=veps[:, 0:np_], in0=sumsq[:, 0:np_],
                    scalar=1.0 / 96, op0=OP.mult,
                    in1=m2[:, 0:np_], op1=OP.subtract)
                nc.vector.tensor_scalar_add(veps[:, 0:np_], veps[:, 0:np_],
                                            EPS)
                lnv = statp.tile([128, 49], F32, name=f"lv_{name}{qi}",
                                 tag="lnv")
                nc.scalar.activation(out=lnv[:, 0:np_], in_=veps[:, 0:np_],
                                     func=AF.Ln)
                rstd = statp.tile([128, 49], F32, name=f"rs_{name}{qi}",
                                  tag="rstd")
                nc.scalar.activation(out=rstd[:, 0:np_], in_=lnv[:, 0:np_],
                                     func=AF.Exp, scale=-0.5)
                nc.vector.scalar_tensor_tensor(
                    out=lnt[:, p0:p1, 0:96], in0=src[:, p0:p1, :],
                    scalar=1.0, op0=OP.mult,
                    in1=m[:, 0:np_, None].to_broadcast([128, np_, 96]),
                    op1=OP.subtract)
                nc.vector.scalar_tensor_tensor(
                    out=lnt[:, p0:p1, 0:96], in0=lnt[:, p0:p1, 0:96],
                    scalar=1.0, op0=OP.mult,
                    in1=rstd[:, 0:np_, None].to_broadcast([128, np_, 96]),
                    op1=OP.mult)
            return lnt

        def transpose_to_T(lnt, dstT):
            """Quartered DMA block-transposes: lnt [128, 49, 128] ->
            dstT[:, 0:6272] position-major."""
            for (p0, p1) in QBOUNDS:
                nc.sync.dma_start_transpose(
                    dstT[:, 128 * p0:128 * p1].rearrange(
                        "a (p w) -> a p w", w=128),
                    lnt[:, p0:p1, :].rearrange("a b c -> a (b c)"))

        def transpose_to_tok(srcT, dst_tok):
            """DMA block-transpose: srcT [96, 6272] -> dst_tok [128, 49, 96]."""
            nc.sync.dma_start_transpose(dst_tok, srcT[0:96, 0:6272])

        eye_b = sb_c["eye49"][:, None, :].to_broadcast([49, 8, 49])

        # ---------------- PHASE A ----------------
        st_a = {}

        def prep_a(c):
            x_tok = xp.tile([128, 49, 96], F32, name="x_tok", tag="x_tok")
            for wr_l in range(4):
                nc.sync.dma_start(out=x_tok[32 * wr_l:32 * wr_l + 32, :, :],
                                  in_=dram_win_ap(xin, c, wr_l))
            ln_x = ln_layer(x_tok, f"a{c}")
            ln_xT = ltp.tile([128, 8192], BF16, name="ln_xT", tag="ln_xT")
            nc.gpsimd.memset(ln_xT[0:97, 6272:8192], 0.0)
            transpose_to_T(ln_x, ln_xT)
            nc.sync.dma_start(out=ln_xT[96:97, 0:6272],
                              in_=cd["ones_row"][0:1, 0:6272])
            st_a[c] = (x_tok, ln_xT)

        def phase_a(c):
            x_tok, ln_xT = st_a.pop(c)
            ln_xT_w = ln_xT.rearrange("k (p w) -> k w p", w=128)

            qT = qkp.tile([128, 6272], BF16, name="qT", tag="qT")
            kT = qkp.tile([128, 6272], BF16, name="kT", tag="kT")
            for n0 in range(0, 6272, 448):
                qps = ps_m.tile([128, 448], F32, name="qps", tag="mm")
                nc.tensor.matmul(qps, sb_c["wqT"][0:97, :],
                                 ln_xT[0:97, n0:n0 + 448],
                                 start=True, stop=True)
                nc.vector.tensor_copy(out=qT[:, n0:n0 + 448], in_=qps)
                kps = ps_m.tile([128, 448], F32, name="kps", tag="mm")
                nc.tensor.matmul(kps, sb_c["wkT"][0:97, :],
                                 ln_xT[0:97, n0:n0 + 448],
                                 start=True, stop=True)
                nc.scalar.copy(out=kT[:, n0:n0 + 448], in_=kps)

            qT_w = qT.rearrange("k (p w) -> k w p", w=128)
            kT_w = kT.rearrange("k (p w) -> k w p", w=128)

            attn_T = aop.tile([96, 6272], BF16, name="attn_T", tag="attn_T")
            attn_T_w = attn_T.rearrange("a (p w) -> a p w", w=128)

            v_tiles = []
            for vt in range(32):
                vps = ps_a.tile([128, 264], F32, name="vps", tag="av")
                for dpq in range(2):
                    for par in range(2):
                        wloc = 2 * (2 * vt + dpq) + par
                        nc.tensor.matmul(
                            vps[64 * par:64 * par + 64,
                                132 * dpq:132 * dpq + 132],
                            ln_xT_w[0:97, wloc, :],
                            sb_c["wv_augT"][0:97, :],
                            start=True, stop=True, skip_group_check=True)
                v_sb = vap.tile([128, 264], BF16, name="v_sb", tag="v_sb")
                nc.scalar.copy(out=v_sb, in_=vps)
                v_tiles.append(v_sb)

            for g in range(8):
                w0 = 16 * g
                exp_ts = []
                for hh in (0, 2):
                    sps = ps_s.tile([128, 1024], F32, name="sps", tag="sps")
                    sps_r = sps.rearrange("a (r x) -> a r x", x=512)
                    for r in range(2):
                        h = hh + r
                        for par in range(2):
                            nc.tensor.matmul(
                                sps_r[64 * par:64 * par + 64, r, 0:392],
                                sb_c["bias_tbl"][:, h, :], eye_b,
                                start=True, stop=False, skip_group_check=True)
                        for j in range(8):
                            for par in range(2):
                                wloc = w0 + 2 * j + par
                                nc.tensor.matmul(
                                    sps_r[64 * par:64 * par + 49, r,
                                          49 * j:49 * j + 49],
                                    kT_w[32 * h:32 * h + 32, wloc, 0:49],
                                    qT_w[32 * h:32 * h + 32, wloc, 0:49],
                                    start=False, stop=True,
                                    tile_position=(32 * h, 64 * par),
                                    skip_group_check=True)
                    exp_sb = exp_p.tile([128, 2, 392], BF16, name="exp_sb",
                                        tag="exp_sb")
                    nc.scalar.activation(out=exp_sb, in_=sps_r[:, :, 0:392],
                                         func=AF.Exp)
                    exp_ts.append(exp_sb)

                o_norm_g = onp.tile([128, 1024], BF16, name="o_norm_g",
                                    tag="o_norm")
                for q2 in range(4):
                    avp = ps_a.tile([128, 264], F32, name="avp", tag="av")
                    avp_b = avp.rearrange("a (b c) -> a b c", c=33)
                    for dpq in range(2):
                        j = 2 * q2 + dpq
                        for h in range(4):
                            e_sb = exp_ts[h // 2]
                            r = h % 2
                            for par in range(2):
                                nc.tensor.matmul(
                                    avp[64 * par:64 * par + 49,
                                        132 * dpq + 33 * h:
                                        132 * dpq + 33 * h + 33],
                                    e_sb[64 * par:64 * par + 64, r,
                                         49 * j:49 * j + 49],
                                    v_tiles[4 * g + q2][64 * par:64 * par + 64,
                                                132 * dpq + 33 * h:
                                                132 * dpq + 33 * h + 33],
                                    start=True, stop=True,
                                    skip_group_check=True)
                    rc = statp.tile([128, 8], F32, name="rc", tag="rc")
                    nc.vector.reciprocal(
                        rc.rearrange("a (b c) -> a b c", c=1),
                        avp_b[:, :, 32:33])
                    nc.vector.scalar_tensor_tensor(
                        out=o_norm_g[:, 256 * q2:256 * q2 + 256].rearrange(
                            "a (b c) -> a b c", c=32),
                        in0=avp_b[:, :, 0:32], scalar=1.0, op0=OP.mult,
                        in1=rc[:, :, None].to_broadcast([128, 8, 32]),
                        op1=OP.mult)

                # one block-transpose + two 4-pair w_out matmuls per group
                oT_sb = otp.tile([128, 1024], BF16, name="oT_sb", tag="oT_sb")
                nc.sync.dma_start_transpose(
                    oT_sb.rearrange("a (r c) -> a r c", c=128),
                    o_norm_g[:, :])
                for half in range(2):
                    wop = ps_m.tile([96, 392], F32, name="wop", tag="mm")
                    nc.tensor.matmul(
                        wop, sb_c["w_outT"],
                        oT_sb.rearrange("a (r m q) -> a r m q",
                                        r=8, q=64)[:, 4 * half:4 * half + 4,
                                                   :, 0:49],
                        start=True, stop=True)
                    wbase = 2 * (8 * g + 4 * half)
                    nc.vector.tensor_scalar_add(
                        attn_T_w[:, :, wbase:wbase + 8].transpose([0, 2, 1]),
                        wop.rearrange("a (r m q) -> a (r m) q", r=4, m=2),
                        sb_c["b_out"][0:96, 0:1])

            attn_tok = tokp.tile([128, 49, 96], BF16, name="attn_tok",
                                 tag="tok_b")
            transpose_to_tok(attn_T, attn_tok)
            y_tok = yp.tile([128, 49, 96], F32, name="y_tok", tag="y_tok")
            nc.vector.scalar_tensor_tensor(
                out=y_tok, in0=attn_tok, scalar=1.0, op0=OP.mult,
                in1=x_tok, op1=OP.add)
            nc.sync.dma_start(out=y_d[128 * c:128 * c + 128, :, :], in_=y_tok)

        # ---------------- PHASE B ----------------
        st_b = {}

        def prep_b(c):
            y_in = xp.tile([128, 49, 96], F32, name="y_in", tag="x_tok")
            nc.sync.dma_start(out=y_in, in_=y_d[128 * c:128 * c + 128, :, :])
            ln2 = ln_layer(y_in, f"b{c}")
            ln2T = ltp.tile([128, 6272], BF16, name="ln2T", tag="ln_xT")
            transpose_to_T(ln2, ln2T)
            nc.sync.dma_start(out=ln2T[96:97, 0:6272],
                              in_=cd["ones_row"][0:1, 0:6272])
            st_b[c] = (y_in, ln2T)

        def phase_b(c):
            y_in, ln2T = st_b.pop(c)
            ffn_T = aop.tile([96, 6272], BF16, name="ffn_T", tag="ffn_T")
            for n0 in range(0, 6272, 448):
                h1 = h1p.tile([128, 3, 448], BF16, name="h1", tag="h1")
                for m in range(3):
                    fps = ps_m.tile([128, 448], F32, name="fps", tag="mm")
                    nc.tensor.matmul(
                        fps, sb_c["w1_augT"][0:97, 128 * m:128 * m + 128],
                        ln2T[0:97, n0:n0 + 448], start=True, stop=True)
                    nc.scalar.activation(out=h1[:, m, :], in_=fps,
                                         func=AF.Gelu)
                f2 = ps_m.tile([96, 448], F32, name="f2", tag="mm")
                for m in range(3):
                    nc.tensor.matmul(f2, sb_c["w2T"][:, m, :], h1[:, m, :],
                                     start=(m == 0), stop=(m == 2),
                                     skip_group_check=True)
                if (n0 // 448) % 2 == 0:
                    nc.vector.tensor_scalar_add(ffn_T[0:96, n0:n0 + 448], f2,
                                                sb_c["b2"][0:96, 0:1])
                else:
                    nc.scalar.activation(out=ffn_T[0:96, n0:n0 + 448], in_=f2,
                                         func=AF.Identity,
                                         bias=sb_c["b2"][0:96, 0:1], scale=1.0)

            ffn_tok = tokp.tile([128, 49, 96], BF16, name="ffn_tok",
                                tag="tok_b")
            transpose_to_tok(ffn_T, ffn_tok)
            for (p0, p1) in ((0, 28), (28, 49)):
                out_tok = outp.tile([128, 28, 96], F32, name="out_tok",
                                    tag="out_tok")
                nc.vector.scalar_tensor_tensor(
                    out=out_tok[:, 0:p1 - p0, :],
                    in0=ffn_tok[:, p0:p1, :], scalar=1.0, op0=OP.mult,
                    in1=y_in[:, p0:p1, :], op1=OP.add)
                for wr_l in range(4):
                    wr = 4 * c + wr_l
                    base = (224 * 7 * wr + 224 * (p0 // 7)) * 96
                    dst = bass.AP(tensor=out_d, offset=base,
                                  ap=[[7 * 96, 32],
                                      [224 * 96, (p1 - p0) // 7],
                                      [1, 7 * 96]])
                    nc.sync.dma_start(
                        out=dst,
                        in_=out_tok[32 * wr_l:32 * wr_l + 32, 0:p1 - p0, :])

        prep_a(0)
        for c in range(n_chunks):
            if c + 1 < n_chunks:
                prep_a(c + 1)
            phase_a(c)
        prep_b(0)
        for c in range(n_chunks):
            if c + 1 < n_chunks:
                prep_b(c + 1)
            phase_b(c)

        for p in reversed(pools):
            p.release()

    _split_multiwaits(nc)
    return nc, consts


def kernel(x, w_qkv, w_out, b_out, rel_bias, ln1_g, ln1_b, ln2_g, ln2_b,
           w1, b1, w2, b2):
    from concourse.bass_utils import run_bass_kernel_spmd

    if "nc" not in _CTX:
        weights = dict(w_qkv=w_qkv, w_out=w_out, b_out=b_out,
                       rel_bias=rel_bias, ln1_g=ln1_g, ln1_b=ln1_b,
                       ln2_g=ln2_g, ln2_b=ln2_b, w1=w1, b1=b1, w2=w2, b2=b2)
        nc, consts = _build(weights)
        _CTX["nc"] = nc
        _CTX["consts"] = consts

    x = np.asarray(x, np.float32)
    in_maps = []
    for i in range(B):
        m = {"x": np.ascontiguousarray(x[i])}
        m.update(_CTX["consts"])
        in_maps.append(m)
    _CTX["in_maps"] = in_maps

    res = run_bass_kernel_spmd(_CTX["nc"], in_maps, core_ids=list(range(B)))
    global LAST_EXEC_NS
    if res.exec_time_ns:
        LAST_EXEC_NS = res.exec_time_ns
    out = np.stack([np.asarray(res.results[i]["out"]) for i in range(B)])
    return out.astype(np.float32)
